# revision 22
# baseline (speedup 1.0000x reference)
"""KANLinear TRN2 Bass kernel (8-core SPMD, token-data-parallel).

Math (matches the jax reference exactly, up to fp rounding):
  y[b,o] = silu(x)[b,:] @ scale_base.T  +  sum_{i,g} B_g(x[b,i]) * w[o,i,g]
with cubic B-spline bases on the uniform grid t_j = -1.75 + 0.25*j
(j = 0..14, 11 bases). On-device identity (truncated-power form): with
  xh  = clamp(x, -1.75, 1.75)
  c_j = relu(4*xh + 7 - j)^3
the basis is the exact 4th difference
  6*B_g(x) = c_g - 4*c_{g+1} + 6*c_{g+2} - 4*c_{g+3} + c_{g+4}.
The 4th difference is split: the device computes the 2nd difference
  d_j = c_j - 2*c_{j+1} + c_{j+2}   (j = 0..12, c_14 == 0)
and the remaining 2nd difference (plus the 1/6) is folded into the
host-prepared weights:
  w2[o,i,j] = (w[o,i,j] - 2*w[o,i,j-1] + w[o,i,j-2]) / 6.
This cuts the on-device DVE combine from 5 passes to 2 at the cost of
13 matmul K-channels instead of 11. d ranges up to ~78, so the matmul
operands use fp16 (not bf16): the fold amplifies lhsT quantization by
the d-magnitude, and bf16's 8-bit mantissa would blow the error budget
(measured 2.3e-2 rel) while fp16 lands at ~2.8e-3.

The x-clamp keeps |arguments| <= 14 (bounds fp32 cancellation error on
d) and reproduces the reference's all-zero basis rows outside the grid
exactly.

Pipeline structure (latency engineering):
 - each in-dim tile's channel work is split into two 7-channel pieces
   so the first 5 d-channels (and their matmuls) unblock after roughly
   half the per-tile ACT+DVE chain;
 - PSUM drains are deferred: emitted (split ACT/DVE) in the middle of
   the NEXT half's first in-dim tile, so bank turnaround costs ~2us of
   PE idle instead of ~6;
 - silu/clamp ACT ops are interleaved per in-dim tile (all the ACT
   functions used live in the one `silu_and_others` table set, so
   interleaving triggers no table reloads).

Sharding: tokens (8192) split 1024/core across 8 cores; grid/coeff/
scale_base replicated (coeff pre-folded and pre-transposed on host to
the matmul K-order k = it*1664 + j*128 + p, i.e. [it, j, p, o]).

Per core the main einsum is a [1024 x 13312] @ [13312 x 1024] matmul
in fp16 (fp32 PSUM accumulation), fed by on-device computed d tiles;
the silu base matmul accumulates into the same PSUM banks.
"""

import numpy as np

import concourse.bass as bass
import concourse.mybir as mybir
import concourse.tile as tile
from concourse import bacc
from concourse.alu_op_type import AluOpType
from concourse.bass_utils import run_bass_kernel_spmd

AF = mybir.ActivationFunctionType
F32 = mybir.dt.float32
F16 = mybir.dt.float16

# problem constants (hardcoded per the task contract)
TOKENS, IN_DIM, OUT_DIM = 8192, 1024, 1024
GRID_SIZE, K = 8, 3
NCHAN = GRID_SIZE + 2 * K + 1  # 15 truncated-power channels
NCH = NCHAN - 1  # 14 nonzero channels (channel 14 is identically 0)
ND = 13  # 2nd-difference channels d_0..d_12
N_CORES = 8
TPC = TOKENS // N_CORES  # tokens per core (1024)
HALF = 512  # tokens per processing chunk (PSUM-bank limited)
NIT = IN_DIM // 128  # in-dim tiles (8)
M_TILES = HALF // 128  # token tiles per half (4)
N_OC = OUT_DIM // 512  # out-dim chunks (2)

X_CLAMP = 1.75
NCA = 7  # channels in piece A (0..6); piece B is 7..13
NDA = 5  # d-channels in chunk A (0..4); chunk B is 5..12

_CACHED = None


def _build_bass():
    nc = bacc.Bacc("TRN2", target_bir_lowering=False, debug=False,
                   num_devices=N_CORES)
    xt = nc.declare_dram_parameter("xt", [IN_DIM, TPC], F32, isOutput=False)
    w2 = nc.declare_dram_parameter("w2", [ND * IN_DIM, OUT_DIM], F16,
                                   isOutput=False)
    sbt = nc.declare_dram_parameter("sbt", [IN_DIM, OUT_DIM], F16,
                                    isOutput=False)
    y = nc.declare_dram_parameter("y", [TPC, OUT_DIM], F32, isOutput=True)

    with tile.TileContext(nc) as tc:
        with (
            tc.tile_pool(name="xts", bufs=3) as xpool,
            tc.tile_pool(name="silu", bufs=4) as spool,
            tc.tile_pool(name="cbuf", bufs=3) as cpool,
            tc.tile_pool(name="sq", bufs=2) as sqpool,
            tc.tile_pool(name="ctmp", bufs=1) as tpool,
            tc.tile_pool(name="dbuf", bufs=3) as dpool,
            tc.tile_pool(name="wts", bufs=3) as wpool,
            tc.tile_pool(name="outs", bufs=4) as opool,
            tc.tile_pool(name="consts", bufs=1) as kpool,
            tc.tile_pool(name="psum", bufs=8, space="PSUM") as ppool,
        ):
            bias_tile = kpool.tile([128, NCH + 1], F32, tag="bias")
            for j in range(NCH):
                # q_j = relu(-4*r + (14-j)); r = relu(1.75 - x)
                nc.vector.memset(bias_tile[:, j:j + 1], float(14 - j))
            nc.vector.memset(bias_tile[:, NCH:NCH + 1], X_CLAMP)
            # prime the ACT table with a dummy Silu before any input lands:
            # the first ACT op determines the loaded table set, and
            # `silu_and_others` also covers relu/square/copy, so every
            # later op runs without a table switch
            prime = kpool.tile([128, 1], F32, tag="prime")
            nc.scalar.activation(prime, bias_tile[:, 0:1], AF.Silu)

            def emit_drains(psums, t_base):
                # split the 8 bank drains across ACT and DVE so the
                # serial drain latency at a half boundary halves
                for oc in range(N_OC):
                    for m in range(M_TILES):
                        ot = opool.tile([128, 512], F32, tag="out")
                        if m % 2 == 0:
                            nc.scalar.copy(ot, psums[oc][m])
                        else:
                            nc.vector.tensor_copy(ot, psums[oc][m])
                        r0 = t_base + m * 128
                        nc.sync.dma_start(
                            out=y[r0:r0 + 128, oc * 512:(oc + 1) * 512],
                            in_=ot)

            pending_drain = None
            for half in range(2):
                t0 = half * HALF
                xt_tiles = []
                for it in range(NIT):
                    xtt = xpool.tile([128, HALF], F32, tag="xt")
                    nc.sync.dma_start(out=xtt,
                                      in_=xt[it * 128:(it + 1) * 128,
                                             t0:t0 + HALF])
                    xt_tiles.append(xtt)

                psums = [[ppool.tile([128, 512], F32, tag="ps",
                                     name=f"ps_{half}_{_oc}_{_m}")
                          for _m in range(M_TILES)] for _oc in range(N_OC)]

                kt_idx = 0
                n_kt = NIT * ND + NIT
                WA = NCA * HALF  # 3584: channel piece boundary
                DA = NDA * HALF  # 2560: d chunk boundary
                WD = ND * HALF  # 6656
                W12 = (ND - 1) * HALF  # 6144: d_12 starts here
                for it in range(NIT):
                    # one-sided clamp via ACT: r = relu(1.75 - x), so that
                    # relu(-4r + (14-j)) == relu(4*min(x,1.75) + 7 - j)
                    rt = spool.tile([128, HALF], F32, tag="rt")
                    nc.scalar.activation(rt, xt_tiles[it], AF.Relu,
                                         bias=bias_tile[:, NCH:NCH + 1],
                                         scale=-1.0)
                    c = cpool.tile([128, NCH * HALF], F32, tag="c")
                    d = dpool.tile([128, WD], F16, tag="d")
                    st = None
                    if it == 0:
                        # silu early: its K-tile leads this half's matmuls
                        st = spool.tile([128, HALF], F16, tag="silu")
                        nc.scalar.activation(st, xt_tiles[it], AF.Silu)
                    # Channel pieces: cube via ACT square + DVE mul, then
                    # the on-device 2nd difference for the d-channels whose
                    # taps are fully cubed so far. GPSIMD is kept idle: any
                    # Pool op locks the SBUF port pair that DVE's 2-input
                    # ops need, fully blocking them. The kernel's first
                    # tile is split finest so the PE start latency is a
                    # quarter-tile of ACT+DVE work; steady-state tiles use
                    # two pieces (enough to overlap, fewest op overheads).
                    if half == 0 and it == 0:
                        pieces = [(0, 3), (3, 5), (5, NCA), (NCA, 11),
                                  (11, NCH)]
                    else:
                        pieces = [(0, NCA), (NCA, NCH)]
                    for (j0, j1) in pieces:
                        for j in range(j0, j1):
                            nc.scalar.activation(
                                c[:, j * HALF:(j + 1) * HALF], rt,
                                AF.Relu, bias=bias_tile[:, j:j + 1],
                                scale=-4.0)
                        w0, w1 = j0 * HALF, j1 * HALF
                        sq = sqpool.tile([128, w1 - w0], F32, tag="sq",
                                         name=f"sq_{half}_{it}_{j0}")
                        nc.scalar.activation(sq, c[:, w0:w1], AF.Square)
                        nc.vector.tensor_mul(c[:, w0:w1], sq, c[:, w0:w1])
                        # d_j = c_j - 2*c_{j+1} + c_{j+2} for j0-2 <= j < j1-2
                        d0 = max(0, j0 - 2) * HALF
                        d1 = (j1 - 2) * HALF
                        tmp = tpool.tile([128, d1 - d0], F32, tag="tA",
                                         name=f"tmp_{half}_{it}_{j0}")
                        nc.vector.scalar_tensor_tensor(
                            tmp, c[:, HALF + d0:HALF + d1], -2.0,
                            c[:, d0:d1], AluOpType.mult, AluOpType.add)
                        nc.vector.tensor_add(
                            d[:, d0:d1], tmp,
                            c[:, 2 * HALF + d0:2 * HALF + d1])
                    # silu for this tile's base matmul (emitted after the
                    # latency-critical squares; it==0 emitted it early)
                    if st is None:
                        st = spool.tile([128, HALF], F16, tag="silu")
                        nc.scalar.activation(st, xt_tiles[it], AF.Silu)
                    # d_12 = c_12 - 2*c_13 (c_14 == 0): single DVE op
                    # writing the fp16 channel directly
                    nc.vector.scalar_tensor_tensor(
                        d[:, W12:], c[:, HALF + W12:HALF + WD], -2.0,
                        c[:, W12:WD], AluOpType.mult, AluOpType.add)
                    # for the first tile of each half, run the silu base
                    # K-tile FIRST: silu is ready right after the x DMA,
                    # long before the first d chunk, so it covers the PE
                    # head (half 0) and the drain window (half 1)
                    silu_first = (it == 0)
                    if silu_first:
                        sbtt = wpool.tile([128, OUT_DIM], F16, tag="sbt")
                        nc.sync.dma_start(out=sbtt,
                                          in_=sbt[it * 128:(it + 1) * 128, :])
                        for oc in range(N_OC):
                            for m in range(M_TILES):
                                nc.tensor.matmul(
                                    psums[oc][m],
                                    lhsT=st[:, m * 128:m * 128 + 128],
                                    rhs=sbtt[:, oc * 512:(oc + 1) * 512],
                                    start=(kt_idx == 0),
                                    stop=False)
                        kt_idx += 1
                    # spline matmul K-tiles, chunk A first
                    for g in range(ND):
                        w2t = wpool.tile([128, OUT_DIM], F16, tag="w2")
                        row = (it * ND + g) * 128
                        nc.sync.dma_start(out=w2t, in_=w2[row:row + 128, :])
                        for oc in range(N_OC):
                            for m in range(M_TILES):
                                nc.tensor.matmul(
                                    psums[oc][m],
                                    lhsT=d[:, g * HALF + m * 128:
                                           g * HALF + m * 128 + 128],
                                    rhs=w2t[:, oc * 512:(oc + 1) * 512],
                                    start=(kt_idx == 0),
                                    stop=False)
                        kt_idx += 1
                        if g == NDA - 1 and it == 0 and pending_drain:
                            # previous half's PSUM drain, overlapped with
                            # this half's first tile of compute
                            emit_drains(*pending_drain)
                            pending_drain = None
                    # this tile's silu base matmul K-tile, accumulated into
                    # the same banks (uniform 14-K-tile PE cadence per tile)
                    if not silu_first:
                        sbtt = wpool.tile([128, OUT_DIM], F16, tag="sbt")
                        nc.sync.dma_start(out=sbtt,
                                          in_=sbt[it * 128:(it + 1) * 128, :])
                        for oc in range(N_OC):
                            for m in range(M_TILES):
                                nc.tensor.matmul(
                                    psums[oc][m],
                                    lhsT=st[:, m * 128:m * 128 + 128],
                                    rhs=sbtt[:, oc * 512:(oc + 1) * 512],
                                    start=False,
                                    stop=(kt_idx == n_kt - 1))
                        kt_idx += 1
                pending_drain = (psums, t0)
            emit_drains(*pending_drain)
    nc.compile()
    return nc


def _prepare_inputs(x, coeff, scale_base, scale_spline):
    x = np.asarray(x, dtype=np.float32)
    coeff = np.asarray(coeff, dtype=np.float32)
    scale_base = np.asarray(scale_base, dtype=np.float32)
    ss = float(np.asarray(scale_spline).reshape(-1)[0])
    # fold the outer 2nd difference (and the 1/6 from the 4th-difference
    # identity) into the weights: w2[o,i,j] = (w[j] - 2w[j-1] + w[j-2])/6
    w = coeff * (ss / 6.0)  # [OUT, IN, 11]
    wf = np.zeros((OUT_DIM, IN_DIM, ND), np.float32)
    wf[..., 0:11] += w
    wf[..., 1:12] += -2.0 * w
    wf[..., 2:13] += w
    # K-order: k = it*1664 + j*128 + p  ->  w2[k, o] = wf[o, it*128+p, j]
    w2 = wf.reshape(OUT_DIM, NIT, 128, ND)
    w2 = np.ascontiguousarray(w2.transpose(1, 3, 2, 0)).reshape(
        ND * IN_DIM, OUT_DIM)
    w2 = w2.astype(np.float16)
    sbt = np.ascontiguousarray(scale_base.T).astype(np.float16)
    in_maps = []
    for c in range(N_CORES):
        xt = np.ascontiguousarray(x[c * TPC:(c + 1) * TPC, :].T)
        in_maps.append({"xt": xt, "w2": w2, "sbt": sbt})
    return in_maps


def _get_bass():
    global _CACHED
    if _CACHED is None:
        _CACHED = _build_bass()
    return _CACHED


def run(inputs, trace=False, **kw):
    nc = _get_bass()
    in_maps = _prepare_inputs(inputs["x"], inputs["coeff"],
                              inputs["scale_base"], inputs["scale_spline"])
    res = run_bass_kernel_spmd(nc, in_maps, list(range(N_CORES)),
                               trace=trace, **kw)
    y = np.concatenate([np.asarray(res.results[c]["y"])
                        for c in range(N_CORES)], axis=0)
    return np.ascontiguousarray(y.astype(np.float32)), res


def kernel(x, grid, coeff, scale_base, scale_spline):
    y, _ = run({"x": x, "grid": grid, "coeff": coeff,
                "scale_base": scale_base, "scale_spline": scale_spline})
    return y


# revision 23
# speedup vs baseline: 1.0175x; 1.0175x over previous
"""KANLinear TRN2 Bass kernel (8-core SPMD, token-data-parallel).

Math (matches the jax reference exactly, up to fp rounding):
  y[b,o] = silu(x)[b,:] @ scale_base.T  +  sum_{i,g} B_g(x[b,i]) * w[o,i,g]
with cubic B-spline bases on the uniform grid t_j = -1.75 + 0.25*j
(j = 0..14, 11 bases). On-device identity (truncated-power form): with
  xh  = clamp(x, -1.75, 1.75)
  c_j = relu(4*xh + 7 - j)^3
the basis is the exact 4th difference
  6*B_g(x) = c_g - 4*c_{g+1} + 6*c_{g+2} - 4*c_{g+3} + c_{g+4}.
The 4th difference is split: the device computes the 2nd difference
  d_j = c_j - 2*c_{j+1} + c_{j+2}   (j = 0..12, c_14 == 0)
and the remaining 2nd difference (plus the 1/6) is folded into the
host-prepared weights:
  w2[o,i,j] = (w[o,i,j] - 2*w[o,i,j-1] + w[o,i,j-2]) / 6.
This cuts the on-device DVE combine from 5 passes to 2 at the cost of
13 matmul K-channels instead of 11. d ranges up to ~78, so the matmul
operands use fp16 (not bf16): the fold amplifies lhsT quantization by
the d-magnitude, and bf16's 8-bit mantissa would blow the error budget
(measured 2.3e-2 rel) while fp16 lands at ~2.8e-3.

The x-clamp keeps |arguments| <= 14 (bounds fp32 cancellation error on
d) and reproduces the reference's all-zero basis rows outside the grid
exactly.

Pipeline structure (latency engineering):
 - each in-dim tile's channel work is split into two 7-channel pieces
   so the first 5 d-channels (and their matmuls) unblock after roughly
   half the per-tile ACT+DVE chain;
 - PSUM drains are deferred: emitted (split ACT/DVE) in the middle of
   the NEXT half's first in-dim tile, so bank turnaround costs ~2us of
   PE idle instead of ~6;
 - silu/clamp ACT ops are interleaved per in-dim tile (all the ACT
   functions used live in the one `silu_and_others` table set, so
   interleaving triggers no table reloads).

Sharding: tokens (8192) split 1024/core across 8 cores; grid/coeff/
scale_base replicated (coeff pre-folded and pre-transposed on host to
the matmul K-order k = it*1664 + j*128 + p, i.e. [it, j, p, o]).

Per core the main einsum is a [1024 x 13312] @ [13312 x 1024] matmul
in fp16 (fp32 PSUM accumulation), fed by on-device computed d tiles;
the silu base matmul accumulates into the same PSUM banks.
"""

import numpy as np

import concourse.bass as bass
import concourse.mybir as mybir
import concourse.tile as tile
from concourse import bacc
from concourse.alu_op_type import AluOpType
from concourse.bass_utils import run_bass_kernel_spmd

AF = mybir.ActivationFunctionType
F32 = mybir.dt.float32
F16 = mybir.dt.float16

# problem constants (hardcoded per the task contract)
TOKENS, IN_DIM, OUT_DIM = 8192, 1024, 1024
GRID_SIZE, K = 8, 3
NCHAN = GRID_SIZE + 2 * K + 1  # 15 truncated-power channels
NCH = NCHAN - 1  # 14 nonzero channels (channel 14 is identically 0)
ND = 13  # 2nd-difference channels d_0..d_12
N_CORES = 8
TPC = TOKENS // N_CORES  # tokens per core (1024)
HALF = 512  # tokens per processing chunk (PSUM-bank limited)
NIT = IN_DIM // 128  # in-dim tiles (8)
M_TILES = HALF // 128  # token tiles per half (4)
N_OC = OUT_DIM // 512  # out-dim chunks (2)

X_CLAMP = 1.75
NCA = 7  # channels in piece A (0..6); piece B is 7..13
NDA = 5  # d-channels in chunk A (0..4); chunk B is 5..12

_CACHED = None


def _build_bass():
    nc = bacc.Bacc("TRN2", target_bir_lowering=False, debug=False,
                   num_devices=N_CORES)
    xt = nc.declare_dram_parameter("xt", [IN_DIM, TPC], F32, isOutput=False)
    w2 = nc.declare_dram_parameter("w2", [ND * IN_DIM, OUT_DIM], F16,
                                   isOutput=False)
    sbt = nc.declare_dram_parameter("sbt", [IN_DIM, OUT_DIM], F16,
                                    isOutput=False)
    y = nc.declare_dram_parameter("y", [TPC, OUT_DIM], F32, isOutput=True)

    with tile.TileContext(nc) as tc:
        with (
            tc.tile_pool(name="xts", bufs=4) as xpool,
            tc.tile_pool(name="silu", bufs=5) as spool,
            tc.tile_pool(name="cbuf", bufs=3) as cpool,
            tc.tile_pool(name="sq", bufs=2) as sqpool,
            tc.tile_pool(name="ctmp", bufs=1) as tpool,
            tc.tile_pool(name="dbuf", bufs=2) as dpool,
            tc.tile_pool(name="wts", bufs=4) as wpool,
            tc.tile_pool(name="outs", bufs=4) as opool,
            tc.tile_pool(name="consts", bufs=1) as kpool,
            tc.tile_pool(name="psum", bufs=8, space="PSUM") as ppool,
        ):
            bias_tile = kpool.tile([128, NCH + 1], F32, tag="bias")
            for j in range(NCH):
                # q_j = relu(-4*r + (14-j)); r = relu(1.75 - x)
                nc.vector.memset(bias_tile[:, j:j + 1], float(14 - j))
            nc.vector.memset(bias_tile[:, NCH:NCH + 1], X_CLAMP)
            # prime the ACT table with a dummy Silu before any input lands:
            # the first ACT op determines the loaded table set, and
            # `silu_and_others` also covers relu/square/copy, so every
            # later op runs without a table switch
            prime = kpool.tile([128, 1], F32, tag="prime")
            nc.scalar.activation(prime, bias_tile[:, 0:1], AF.Silu)

            def emit_drains(psums, t_base):
                # split the 8 bank drains across ACT and DVE so the
                # serial drain latency at a half boundary halves
                for oc in range(N_OC):
                    for m in range(M_TILES):
                        ot = opool.tile([128, 512], F32, tag="out")
                        if m % 2 == 0:
                            nc.scalar.copy(ot, psums[oc][m])
                        else:
                            nc.vector.tensor_copy(ot, psums[oc][m])
                        r0 = t_base + m * 128
                        nc.sync.dma_start(
                            out=y[r0:r0 + 128, oc * 512:(oc + 1) * 512],
                            in_=ot)

            pending_drain = None
            for half in range(2):
                t0 = half * HALF
                xt_tiles = []
                for it in range(NIT):
                    xtt = xpool.tile([128, HALF], F32, tag="xt")
                    nc.sync.dma_start(out=xtt,
                                      in_=xt[it * 128:(it + 1) * 128,
                                             t0:t0 + HALF])
                    xt_tiles.append(xtt)

                psums = [[ppool.tile([128, 512], F32, tag="ps",
                                     name=f"ps_{half}_{_oc}_{_m}")
                          for _m in range(M_TILES)] for _oc in range(N_OC)]

                kt_idx = 0
                n_kt = NIT * ND + NIT
                WA = NCA * HALF  # 3584: channel piece boundary
                DA = NDA * HALF  # 2560: d chunk boundary
                WD = ND * HALF  # 6656
                W12 = (ND - 1) * HALF  # 6144: d_12 starts here
                for it in range(NIT):
                    # one-sided clamp via ACT: r = relu(1.75 - x), so that
                    # relu(-4r + (14-j)) == relu(4*min(x,1.75) + 7 - j)
                    rt = spool.tile([128, HALF], F32, tag="rt")
                    nc.scalar.activation(rt, xt_tiles[it], AF.Relu,
                                         bias=bias_tile[:, NCH:NCH + 1],
                                         scale=-1.0)
                    c = cpool.tile([128, NCH * HALF], F32, tag="c")
                    d = dpool.tile([128, WD], F16, tag="d")
                    st = None
                    if it == 0:
                        # silu early: its K-tile leads this half's matmuls
                        st = spool.tile([128, HALF], F16, tag="silu")
                        nc.scalar.activation(st, xt_tiles[it], AF.Silu)
                    # Channel pieces: cube via ACT square + DVE mul, then
                    # the on-device 2nd difference for the d-channels whose
                    # taps are fully cubed so far. GPSIMD is kept idle: any
                    # Pool op locks the SBUF port pair that DVE's 2-input
                    # ops need, fully blocking them. The kernel's first
                    # tile is split finest so the PE start latency is a
                    # quarter-tile of ACT+DVE work; steady-state tiles use
                    # two pieces (enough to overlap, fewest op overheads).
                    if half == 0 and it == 0:
                        pieces = [(0, 3), (3, 5), (5, NCA), (NCA, 11),
                                  (11, NCH)]
                    else:
                        pieces = [(0, NCA), (NCA, NCH)]
                    for (j0, j1) in pieces:
                        for j in range(j0, j1):
                            nc.scalar.activation(
                                c[:, j * HALF:(j + 1) * HALF], rt,
                                AF.Relu, bias=bias_tile[:, j:j + 1],
                                scale=-4.0)
                        w0, w1 = j0 * HALF, j1 * HALF
                        sq = sqpool.tile([128, w1 - w0], F32, tag="sq",
                                         name=f"sq_{half}_{it}_{j0}")
                        nc.scalar.activation(sq, c[:, w0:w1], AF.Square)
                        nc.vector.tensor_mul(c[:, w0:w1], sq, c[:, w0:w1])
                        # d_j = c_j - 2*c_{j+1} + c_{j+2} for j0-2 <= j < j1-2
                        d0 = max(0, j0 - 2) * HALF
                        d1 = (j1 - 2) * HALF
                        tmp = tpool.tile([128, d1 - d0], F32, tag="tA",
                                         name=f"tmp_{half}_{it}_{j0}")
                        nc.vector.scalar_tensor_tensor(
                            tmp, c[:, HALF + d0:HALF + d1], -2.0,
                            c[:, d0:d1], AluOpType.mult, AluOpType.add)
                        nc.vector.tensor_add(
                            d[:, d0:d1], tmp,
                            c[:, 2 * HALF + d0:2 * HALF + d1])
                    # silu for this tile's base matmul (emitted after the
                    # latency-critical squares; it==0 emitted it early)
                    if st is None:
                        st = spool.tile([128, HALF], F16, tag="silu")
                        nc.scalar.activation(st, xt_tiles[it], AF.Silu)
                    # d_12 = c_12 - 2*c_13 (c_14 == 0): single DVE op
                    # writing the fp16 channel directly
                    nc.vector.scalar_tensor_tensor(
                        d[:, W12:], c[:, HALF + W12:HALF + WD], -2.0,
                        c[:, W12:WD], AluOpType.mult, AluOpType.add)
                    # for the first tile of each half, run the silu base
                    # K-tile FIRST: silu is ready right after the x DMA,
                    # long before the first d chunk, so it covers the PE
                    # head (half 0) and the drain window (half 1)
                    silu_first = (it == 0)
                    if silu_first:
                        sbtt = wpool.tile([128, OUT_DIM], F16, tag="sbt")
                        nc.sync.dma_start(out=sbtt,
                                          in_=sbt[it * 128:(it + 1) * 128, :])
                        for oc in range(N_OC):
                            for m in range(M_TILES):
                                nc.tensor.matmul(
                                    psums[oc][m],
                                    lhsT=st[:, m * 128:m * 128 + 128],
                                    rhs=sbtt[:, oc * 512:(oc + 1) * 512],
                                    start=(kt_idx == 0),
                                    stop=False)
                        kt_idx += 1
                    # spline matmul K-tiles, chunk A first
                    for g in range(ND):
                        w2t = wpool.tile([128, OUT_DIM], F16, tag="w2")
                        row = (it * ND + g) * 128
                        nc.sync.dma_start(out=w2t, in_=w2[row:row + 128, :])
                        for oc in range(N_OC):
                            for m in range(M_TILES):
                                nc.tensor.matmul(
                                    psums[oc][m],
                                    lhsT=d[:, g * HALF + m * 128:
                                           g * HALF + m * 128 + 128],
                                    rhs=w2t[:, oc * 512:(oc + 1) * 512],
                                    start=(kt_idx == 0),
                                    stop=False)
                        kt_idx += 1
                        if g == NDA - 1 and it == 0 and pending_drain:
                            # previous half's PSUM drain, overlapped with
                            # this half's first tile of compute
                            emit_drains(*pending_drain)
                            pending_drain = None
                    # this tile's silu base matmul K-tile, accumulated into
                    # the same banks (uniform 14-K-tile PE cadence per tile)
                    if not silu_first:
                        sbtt = wpool.tile([128, OUT_DIM], F16, tag="sbt")
                        nc.sync.dma_start(out=sbtt,
                                          in_=sbt[it * 128:(it + 1) * 128, :])
                        for oc in range(N_OC):
                            for m in range(M_TILES):
                                nc.tensor.matmul(
                                    psums[oc][m],
                                    lhsT=st[:, m * 128:m * 128 + 128],
                                    rhs=sbtt[:, oc * 512:(oc + 1) * 512],
                                    start=False,
                                    stop=(kt_idx == n_kt - 1))
                        kt_idx += 1
                pending_drain = (psums, t0)
            emit_drains(*pending_drain)
    nc.compile()
    return nc


def _prepare_inputs(x, coeff, scale_base, scale_spline):
    x = np.asarray(x, dtype=np.float32)
    coeff = np.asarray(coeff, dtype=np.float32)
    scale_base = np.asarray(scale_base, dtype=np.float32)
    ss = float(np.asarray(scale_spline).reshape(-1)[0])
    # fold the outer 2nd difference (and the 1/6 from the 4th-difference
    # identity) into the weights: w2[o,i,j] = (w[j] - 2w[j-1] + w[j-2])/6
    w = coeff * (ss / 6.0)  # [OUT, IN, 11]
    wf = np.zeros((OUT_DIM, IN_DIM, ND), np.float32)
    wf[..., 0:11] += w
    wf[..., 1:12] += -2.0 * w
    wf[..., 2:13] += w
    # K-order: k = it*1664 + j*128 + p  ->  w2[k, o] = wf[o, it*128+p, j]
    w2 = wf.reshape(OUT_DIM, NIT, 128, ND)
    w2 = np.ascontiguousarray(w2.transpose(1, 3, 2, 0)).reshape(
        ND * IN_DIM, OUT_DIM)
    w2 = w2.astype(np.float16)
    sbt = np.ascontiguousarray(scale_base.T).astype(np.float16)
    in_maps = []
    for c in range(N_CORES):
        xt = np.ascontiguousarray(x[c * TPC:(c + 1) * TPC, :].T)
        in_maps.append({"xt": xt, "w2": w2, "sbt": sbt})
    return in_maps


def _get_bass():
    global _CACHED
    if _CACHED is None:
        _CACHED = _build_bass()
    return _CACHED


def run(inputs, trace=False, **kw):
    nc = _get_bass()
    in_maps = _prepare_inputs(inputs["x"], inputs["coeff"],
                              inputs["scale_base"], inputs["scale_spline"])
    res = run_bass_kernel_spmd(nc, in_maps, list(range(N_CORES)),
                               trace=trace, **kw)
    y = np.concatenate([np.asarray(res.results[c]["y"])
                        for c in range(N_CORES)], axis=0)
    return np.ascontiguousarray(y.astype(np.float32)), res


def kernel(x, grid, coeff, scale_base, scale_spline):
    y, _ = run({"x": x, "grid": grid, "coeff": coeff,
                "scale_base": scale_base, "scale_spline": scale_spline})
    return y


# revision 24
# speedup vs baseline: 1.0296x; 1.0118x over previous
"""KANLinear TRN2 Bass kernel (8-core SPMD, token-data-parallel).

Math (matches the jax reference exactly, up to fp rounding):
  y[b,o] = silu(x)[b,:] @ scale_base.T  +  sum_{i,g} B_g(x[b,i]) * w[o,i,g]
with cubic B-spline bases on the uniform grid t_j = -1.75 + 0.25*j
(j = 0..14, 11 bases). On-device identity (truncated-power form): with
  xh  = clamp(x, -1.75, 1.75)
  c_j = relu(4*xh + 7 - j)^3
the basis is the exact 4th difference
  6*B_g(x) = c_g - 4*c_{g+1} + 6*c_{g+2} - 4*c_{g+3} + c_{g+4}.
The 4th difference is split: the device computes the 2nd difference
  d_j = c_j - 2*c_{j+1} + c_{j+2}   (j = 0..12, c_14 == 0)
and the remaining 2nd difference (plus the 1/6) is folded into the
host-prepared weights:
  w2[o,i,j] = (w[o,i,j] - 2*w[o,i,j-1] + w[o,i,j-2]) / 6.
This cuts the on-device DVE combine from 5 passes to 2 at the cost of
13 matmul K-channels instead of 11. d ranges up to ~78, so the matmul
operands use fp16 (not bf16): the fold amplifies lhsT quantization by
the d-magnitude, and bf16's 8-bit mantissa would blow the error budget
(measured 2.3e-2 rel) while fp16 lands at ~2.8e-3.

The x-clamp keeps |arguments| <= 14 (bounds fp32 cancellation error on
d) and reproduces the reference's all-zero basis rows outside the grid
exactly.

Pipeline structure (latency engineering):
 - each in-dim tile's channel work is split into two 7-channel pieces
   so the first 5 d-channels (and their matmuls) unblock after roughly
   half the per-tile ACT+DVE chain;
 - PSUM drains are deferred: emitted (split ACT/DVE) in the middle of
   the NEXT half's first in-dim tile, so bank turnaround costs ~2us of
   PE idle instead of ~6;
 - silu/clamp ACT ops are interleaved per in-dim tile (all the ACT
   functions used live in the one `silu_and_others` table set, so
   interleaving triggers no table reloads).

Sharding: tokens (8192) split 1024/core across 8 cores; grid/coeff/
scale_base replicated (coeff pre-folded and pre-transposed on host to
the matmul K-order k = it*1664 + j*128 + p, i.e. [it, j, p, o]).

Per core the main einsum is a [1024 x 13312] @ [13312 x 1024] matmul
in fp16 (fp32 PSUM accumulation), fed by on-device computed d tiles;
the silu base matmul accumulates into the same PSUM banks.
"""

import numpy as np

import concourse.bass as bass
import concourse.mybir as mybir
import concourse.tile as tile
from concourse import bacc
from concourse.alu_op_type import AluOpType
from concourse.bass_utils import run_bass_kernel_spmd

AF = mybir.ActivationFunctionType
F32 = mybir.dt.float32
F16 = mybir.dt.float16

# problem constants (hardcoded per the task contract)
TOKENS, IN_DIM, OUT_DIM = 8192, 1024, 1024
GRID_SIZE, K = 8, 3
NCHAN = GRID_SIZE + 2 * K + 1  # 15 truncated-power channels
NCH = NCHAN - 1  # 14 nonzero channels (channel 14 is identically 0)
ND = 13  # 2nd-difference channels d_0..d_12
N_CORES = 8
TPC = TOKENS // N_CORES  # tokens per core (1024)
HALF = 512  # tokens per processing chunk (PSUM-bank limited)
NIT = IN_DIM // 128  # in-dim tiles (8)
M_TILES = HALF // 128  # token tiles per half (4)
N_OC = OUT_DIM // 512  # out-dim chunks (2)

X_CLAMP = 1.75
NCA = 7  # channels in piece A (0..6); piece B is 7..13
NDA = 5  # d-channels in chunk A (0..4); chunk B is 5..12

_CACHED = None


def _build_bass():
    nc = bacc.Bacc("TRN2", target_bir_lowering=False, debug=False,
                   num_devices=N_CORES)
    xt = nc.declare_dram_parameter("xt", [IN_DIM, TPC], F32, isOutput=False)
    w2 = nc.declare_dram_parameter("w2", [ND * IN_DIM, OUT_DIM], F16,
                                   isOutput=False)
    sbt = nc.declare_dram_parameter("sbt", [IN_DIM, OUT_DIM], F16,
                                    isOutput=False)
    y = nc.declare_dram_parameter("y", [TPC, OUT_DIM], F32, isOutput=True)

    with tile.TileContext(nc) as tc:
        with (
            tc.tile_pool(name="xts", bufs=4) as xpool,
            tc.tile_pool(name="silu", bufs=5) as spool,
            tc.tile_pool(name="cbuf", bufs=2) as cpool,
            tc.tile_pool(name="sq", bufs=2) as sqpool,
            tc.tile_pool(name="ctmp", bufs=1) as tpool,
            tc.tile_pool(name="dbuf", bufs=3) as dpool,
            tc.tile_pool(name="wts", bufs=4) as wpool,
            tc.tile_pool(name="outs", bufs=4) as opool,
            tc.tile_pool(name="consts", bufs=1) as kpool,
            tc.tile_pool(name="psum", bufs=8, space="PSUM") as ppool,
        ):
            bias_tile = kpool.tile([128, NCH + 1], F32, tag="bias")
            for j in range(NCH):
                # q_j = relu(-4*r + (14-j)); r = relu(1.75 - x)
                nc.vector.memset(bias_tile[:, j:j + 1], float(14 - j))
            nc.vector.memset(bias_tile[:, NCH:NCH + 1], X_CLAMP)
            # prime the ACT table with a dummy Silu before any input lands:
            # the first ACT op determines the loaded table set, and
            # `silu_and_others` also covers relu/square/copy, so every
            # later op runs without a table switch
            prime = kpool.tile([128, 1], F32, tag="prime")
            nc.scalar.activation(prime, bias_tile[:, 0:1], AF.Silu)

            def emit_drains(psums, t_base):
                # split the 8 bank drains across ACT and DVE so the
                # serial drain latency at a half boundary halves
                for oc in range(N_OC):
                    for m in range(M_TILES):
                        ot = opool.tile([128, 512], F32, tag="out")
                        if m % 2 == 0:
                            nc.scalar.copy(ot, psums[oc][m])
                        else:
                            nc.vector.tensor_copy(ot, psums[oc][m])
                        r0 = t_base + m * 128
                        nc.sync.dma_start(
                            out=y[r0:r0 + 128, oc * 512:(oc + 1) * 512],
                            in_=ot)

            pending_drain = None
            for half in range(2):
                t0 = half * HALF
                xt_tiles = []
                for it in range(NIT):
                    xtt = xpool.tile([128, HALF], F32, tag="xt")
                    nc.sync.dma_start(out=xtt,
                                      in_=xt[it * 128:(it + 1) * 128,
                                             t0:t0 + HALF])
                    xt_tiles.append(xtt)

                psums = [[ppool.tile([128, 512], F32, tag="ps",
                                     name=f"ps_{half}_{_oc}_{_m}")
                          for _m in range(M_TILES)] for _oc in range(N_OC)]

                kt_idx = 0
                n_kt = NIT * ND + NIT
                WA = NCA * HALF  # 3584: channel piece boundary
                DA = NDA * HALF  # 2560: d chunk boundary
                WD = ND * HALF  # 6656
                W12 = (ND - 1) * HALF  # 6144: d_12 starts here
                for it in range(NIT):
                    # one-sided clamp via ACT: r = relu(1.75 - x), so that
                    # relu(-4r + (14-j)) == relu(4*min(x,1.75) + 7 - j)
                    rt = spool.tile([128, HALF], F32, tag="rt")
                    nc.scalar.activation(rt, xt_tiles[it], AF.Relu,
                                         bias=bias_tile[:, NCH:NCH + 1],
                                         scale=-1.0)
                    c = cpool.tile([128, NCH * HALF], F32, tag="c")
                    d = dpool.tile([128, WD], F16, tag="d")
                    st = None
                    if it == 0:
                        # silu early: its K-tile leads this half's matmuls
                        st = spool.tile([128, HALF], F16, tag="silu")
                        nc.scalar.activation(st, xt_tiles[it], AF.Silu)
                    # Channel pieces: cube via ACT square + DVE mul, then
                    # the on-device 2nd difference for the d-channels whose
                    # taps are fully cubed so far. GPSIMD is kept idle: any
                    # Pool op locks the SBUF port pair that DVE's 2-input
                    # ops need, fully blocking them. The kernel's first
                    # tile is split finest so the PE start latency is a
                    # quarter-tile of ACT+DVE work; steady-state tiles use
                    # two pieces (enough to overlap, fewest op overheads).
                    if half == 0 and it == 0:
                        pieces = [(0, 3), (3, 5), (5, NCA), (NCA, 11),
                                  (11, NCH)]
                    else:
                        pieces = [(0, NCA), (NCA, NCH)]
                    for (j0, j1) in pieces:
                        for j in range(j0, j1):
                            nc.scalar.activation(
                                c[:, j * HALF:(j + 1) * HALF], rt,
                                AF.Relu, bias=bias_tile[:, j:j + 1],
                                scale=-4.0)
                        w0, w1 = j0 * HALF, j1 * HALF
                        sq = sqpool.tile([128, w1 - w0], F32, tag="sq",
                                         name=f"sq_{half}_{it}_{j0}")
                        nc.scalar.activation(sq, c[:, w0:w1], AF.Square)
                        nc.vector.tensor_mul(c[:, w0:w1], sq, c[:, w0:w1])
                        # d_j = c_j - 2*c_{j+1} + c_{j+2} for j0-2 <= j < j1-2
                        d0 = max(0, j0 - 2) * HALF
                        d1 = (j1 - 2) * HALF
                        tmp = tpool.tile([128, d1 - d0], F32, tag="tA",
                                         name=f"tmp_{half}_{it}_{j0}")
                        nc.vector.scalar_tensor_tensor(
                            tmp, c[:, HALF + d0:HALF + d1], -2.0,
                            c[:, d0:d1], AluOpType.mult, AluOpType.add)
                        nc.vector.tensor_add(
                            d[:, d0:d1], tmp,
                            c[:, 2 * HALF + d0:2 * HALF + d1])
                    # silu for this tile's base matmul (emitted after the
                    # latency-critical squares; it==0 emitted it early)
                    if st is None:
                        st = spool.tile([128, HALF], F16, tag="silu")
                        nc.scalar.activation(st, xt_tiles[it], AF.Silu)
                    # d_12 = c_12 - 2*c_13 (c_14 == 0): single DVE op
                    # writing the fp16 channel directly
                    nc.vector.scalar_tensor_tensor(
                        d[:, W12:], c[:, HALF + W12:HALF + WD], -2.0,
                        c[:, W12:WD], AluOpType.mult, AluOpType.add)
                    # for the first tile of each half, run the silu base
                    # K-tile FIRST: silu is ready right after the x DMA,
                    # long before the first d chunk, so it covers the PE
                    # head (half 0) and the drain window (half 1)
                    silu_first = (it == 0)
                    if silu_first:
                        sbtt = wpool.tile([128, OUT_DIM], F16, tag="sbt")
                        nc.sync.dma_start(out=sbtt,
                                          in_=sbt[it * 128:(it + 1) * 128, :])
                        for oc in range(N_OC):
                            for m in range(M_TILES):
                                nc.tensor.matmul(
                                    psums[oc][m],
                                    lhsT=st[:, m * 128:m * 128 + 128],
                                    rhs=sbtt[:, oc * 512:(oc + 1) * 512],
                                    start=(kt_idx == 0),
                                    stop=False)
                        kt_idx += 1
                    # spline matmul K-tiles, chunk A first
                    for g in range(ND):
                        w2t = wpool.tile([128, OUT_DIM], F16, tag="w2")
                        row = (it * ND + g) * 128
                        nc.sync.dma_start(out=w2t, in_=w2[row:row + 128, :])
                        for oc in range(N_OC):
                            for m in range(M_TILES):
                                nc.tensor.matmul(
                                    psums[oc][m],
                                    lhsT=d[:, g * HALF + m * 128:
                                           g * HALF + m * 128 + 128],
                                    rhs=w2t[:, oc * 512:(oc + 1) * 512],
                                    start=(kt_idx == 0),
                                    stop=False)
                        kt_idx += 1
                        if g == NDA - 1 and it == 0 and pending_drain:
                            # previous half's PSUM drain, overlapped with
                            # this half's first tile of compute
                            emit_drains(*pending_drain)
                            pending_drain = None
                    # this tile's silu base matmul K-tile, accumulated into
                    # the same banks (uniform 14-K-tile PE cadence per tile)
                    if not silu_first:
                        sbtt = wpool.tile([128, OUT_DIM], F16, tag="sbt")
                        nc.sync.dma_start(out=sbtt,
                                          in_=sbt[it * 128:(it + 1) * 128, :])
                        for oc in range(N_OC):
                            for m in range(M_TILES):
                                nc.tensor.matmul(
                                    psums[oc][m],
                                    lhsT=st[:, m * 128:m * 128 + 128],
                                    rhs=sbtt[:, oc * 512:(oc + 1) * 512],
                                    start=False,
                                    stop=(kt_idx == n_kt - 1))
                        kt_idx += 1
                pending_drain = (psums, t0)
            emit_drains(*pending_drain)
    nc.compile()
    return nc


def _prepare_inputs(x, coeff, scale_base, scale_spline):
    x = np.asarray(x, dtype=np.float32)
    coeff = np.asarray(coeff, dtype=np.float32)
    scale_base = np.asarray(scale_base, dtype=np.float32)
    ss = float(np.asarray(scale_spline).reshape(-1)[0])
    # fold the outer 2nd difference (and the 1/6 from the 4th-difference
    # identity) into the weights: w2[o,i,j] = (w[j] - 2w[j-1] + w[j-2])/6
    w = coeff * (ss / 6.0)  # [OUT, IN, 11]
    wf = np.zeros((OUT_DIM, IN_DIM, ND), np.float32)
    wf[..., 0:11] += w
    wf[..., 1:12] += -2.0 * w
    wf[..., 2:13] += w
    # K-order: k = it*1664 + j*128 + p  ->  w2[k, o] = wf[o, it*128+p, j]
    w2 = wf.reshape(OUT_DIM, NIT, 128, ND)
    w2 = np.ascontiguousarray(w2.transpose(1, 3, 2, 0)).reshape(
        ND * IN_DIM, OUT_DIM)
    w2 = w2.astype(np.float16)
    sbt = np.ascontiguousarray(scale_base.T).astype(np.float16)
    in_maps = []
    for c in range(N_CORES):
        xt = np.ascontiguousarray(x[c * TPC:(c + 1) * TPC, :].T)
        in_maps.append({"xt": xt, "w2": w2, "sbt": sbt})
    return in_maps


def _get_bass():
    global _CACHED
    if _CACHED is None:
        _CACHED = _build_bass()
    return _CACHED


def run(inputs, trace=False, **kw):
    nc = _get_bass()
    in_maps = _prepare_inputs(inputs["x"], inputs["coeff"],
                              inputs["scale_base"], inputs["scale_spline"])
    res = run_bass_kernel_spmd(nc, in_maps, list(range(N_CORES)),
                               trace=trace, **kw)
    y = np.concatenate([np.asarray(res.results[c]["y"])
                        for c in range(N_CORES)], axis=0)
    return np.ascontiguousarray(y.astype(np.float32)), res


def kernel(x, grid, coeff, scale_base, scale_spline):
    y, _ = run({"x": x, "grid": grid, "coeff": coeff,
                "scale_base": scale_base, "scale_spline": scale_spline})
    return y


# revision 25
# speedup vs baseline: 1.0350x; 1.0053x over previous
"""KANLinear TRN2 Bass kernel (8-core SPMD, token-data-parallel).

Math (matches the jax reference exactly, up to fp rounding):
  y[b,o] = silu(x)[b,:] @ scale_base.T  +  sum_{i,g} B_g(x[b,i]) * w[o,i,g]
with cubic B-spline bases on the uniform grid t_j = -1.75 + 0.25*j
(j = 0..14, 11 bases). On-device identity (truncated-power form): with
  xh  = clamp(x, -1.75, 1.75)
  c_j = relu(4*xh + 7 - j)^3
the basis is the exact 4th difference
  6*B_g(x) = c_g - 4*c_{g+1} + 6*c_{g+2} - 4*c_{g+3} + c_{g+4}.
The 4th difference is split: the device computes the 2nd difference
  d_j = c_j - 2*c_{j+1} + c_{j+2}   (j = 0..12, c_14 == 0)
and the remaining 2nd difference (plus the 1/6) is folded into the
host-prepared weights:
  w2[o,i,j] = (w[o,i,j] - 2*w[o,i,j-1] + w[o,i,j-2]) / 6.
This cuts the on-device DVE combine from 5 passes to 2 at the cost of
13 matmul K-channels instead of 11. d ranges up to ~78, so the matmul
operands use fp16 (not bf16): the fold amplifies lhsT quantization by
the d-magnitude, and bf16's 8-bit mantissa would blow the error budget
(measured 2.3e-2 rel) while fp16 lands at ~2.8e-3.

The x-clamp keeps |arguments| <= 14 (bounds fp32 cancellation error on
d) and reproduces the reference's all-zero basis rows outside the grid
exactly.

Pipeline structure (latency engineering):
 - each in-dim tile's channel work is split into two 7-channel pieces
   so the first 5 d-channels (and their matmuls) unblock after roughly
   half the per-tile ACT+DVE chain;
 - PSUM drains are deferred: emitted (split ACT/DVE) in the middle of
   the NEXT half's first in-dim tile, so bank turnaround costs ~2us of
   PE idle instead of ~6;
 - silu/clamp ACT ops are interleaved per in-dim tile (all the ACT
   functions used live in the one `silu_and_others` table set, so
   interleaving triggers no table reloads).

Sharding: tokens (8192) split 1024/core across 8 cores; grid/coeff/
scale_base replicated (coeff pre-folded and pre-transposed on host to
the matmul K-order k = it*1664 + j*128 + p, i.e. [it, j, p, o]).

Per core the main einsum is a [1024 x 13312] @ [13312 x 1024] matmul
in fp16 (fp32 PSUM accumulation), fed by on-device computed d tiles;
the silu base matmul accumulates into the same PSUM banks.
"""

import numpy as np

import concourse.bass as bass
import concourse.mybir as mybir
import concourse.tile as tile
from concourse import bacc
from concourse.alu_op_type import AluOpType
from concourse.bass_utils import run_bass_kernel_spmd

AF = mybir.ActivationFunctionType
F32 = mybir.dt.float32
F16 = mybir.dt.float16

# problem constants (hardcoded per the task contract)
TOKENS, IN_DIM, OUT_DIM = 8192, 1024, 1024
GRID_SIZE, K = 8, 3
NCHAN = GRID_SIZE + 2 * K + 1  # 15 truncated-power channels
NCH = NCHAN - 1  # 14 nonzero channels (channel 14 is identically 0)
ND = 13  # 2nd-difference channels d_0..d_12
N_CORES = 8
TPC = TOKENS // N_CORES  # tokens per core (1024)
HALF = 512  # tokens per processing chunk (PSUM-bank limited)
NIT = IN_DIM // 128  # in-dim tiles (8)
M_TILES = HALF // 128  # token tiles per half (4)
N_OC = OUT_DIM // 512  # out-dim chunks (2)

X_CLAMP = 1.75
NCA = 7  # channels in piece A (0..6); piece B is 7..13
NDA = 5  # d-channels in chunk A (0..4); chunk B is 5..12

_CACHED = None


def _build_bass():
    nc = bacc.Bacc("TRN2", target_bir_lowering=False, debug=False,
                   num_devices=N_CORES)
    xt = nc.declare_dram_parameter("xt", [IN_DIM, TPC], F32, isOutput=False)
    w2 = nc.declare_dram_parameter("w2", [ND * IN_DIM, OUT_DIM], F16,
                                   isOutput=False)
    sbt = nc.declare_dram_parameter("sbt", [IN_DIM, OUT_DIM], F16,
                                    isOutput=False)
    y = nc.declare_dram_parameter("y", [TPC, OUT_DIM], F32, isOutput=True)

    with tile.TileContext(nc) as tc:
        with (
            tc.tile_pool(name="xts", bufs=4) as xpool,
            tc.tile_pool(name="silu", bufs=5) as spool,
            tc.tile_pool(name="cbuf", bufs=2) as cpool,
            tc.tile_pool(name="sq", bufs=2) as sqpool,
            tc.tile_pool(name="ctmp", bufs=1) as tpool,
            tc.tile_pool(name="dbuf", bufs=3) as dpool,
            tc.tile_pool(name="wts", bufs=4) as wpool,
            tc.tile_pool(name="outs", bufs=4) as opool,
            tc.tile_pool(name="consts", bufs=1) as kpool,
            tc.tile_pool(name="psum", bufs=8, space="PSUM") as ppool,
        ):
            bias_tile = kpool.tile([128, NCH + 1], F32, tag="bias")
            for j in range(NCH):
                # q_j = relu(-4*r + (14-j)); r = relu(1.75 - x)
                nc.vector.memset(bias_tile[:, j:j + 1], float(14 - j))
            nc.vector.memset(bias_tile[:, NCH:NCH + 1], X_CLAMP)
            # prime the ACT table with a dummy Silu before any input lands:
            # the first ACT op determines the loaded table set, and
            # `silu_and_others` also covers relu/square/copy, so every
            # later op runs without a table switch
            prime = kpool.tile([128, 1], F32, tag="prime")
            nc.scalar.activation(prime, bias_tile[:, 0:1], AF.Silu)

            def emit_drains(psums, t_base):
                # split the 8 bank drains across ACT and DVE so the
                # serial drain latency at a half boundary halves
                for oc in range(N_OC):
                    for m in range(M_TILES):
                        ot = opool.tile([128, 512], F32, tag="out")
                        if m % 2 == 0:
                            nc.scalar.copy(ot, psums[oc][m])
                        else:
                            nc.vector.tensor_copy(ot, psums[oc][m])
                        r0 = t_base + m * 128
                        nc.sync.dma_start(
                            out=y[r0:r0 + 128, oc * 512:(oc + 1) * 512],
                            in_=ot)

            pending_drain = None
            for half in range(2):
                t0 = half * HALF
                xt_tiles = []
                for it in range(NIT):
                    xtt = xpool.tile([128, HALF], F32, tag="xt")
                    nc.sync.dma_start(out=xtt,
                                      in_=xt[it * 128:(it + 1) * 128,
                                             t0:t0 + HALF])
                    xt_tiles.append(xtt)

                psums = [[ppool.tile([128, 512], F32, tag="ps",
                                     name=f"ps_{half}_{_oc}_{_m}")
                          for _m in range(M_TILES)] for _oc in range(N_OC)]

                kt_idx = 0
                n_kt = NIT * ND + NIT
                WA = NCA * HALF  # 3584: channel piece boundary
                DA = NDA * HALF  # 2560: d chunk boundary
                WD = ND * HALF  # 6656
                W12 = (ND - 1) * HALF  # 6144: d_12 starts here
                for it in range(NIT):
                    # one-sided clamp via ACT: r = relu(1.75 - x), so that
                    # relu(-4r + (14-j)) == relu(4*min(x,1.75) + 7 - j)
                    rt = spool.tile([128, HALF], F32, tag="rt")
                    nc.scalar.activation(rt, xt_tiles[it], AF.Relu,
                                         bias=bias_tile[:, NCH:NCH + 1],
                                         scale=-1.0)
                    c = cpool.tile([128, NCH * HALF], F32, tag="c")
                    d = dpool.tile([128, WD], F16, tag="d")
                    st = None
                    if it == 0:
                        # silu early: its K-tile leads this half's matmuls
                        st = spool.tile([128, HALF], F16, tag="silu")
                        nc.scalar.activation(st, xt_tiles[it], AF.Silu)
                    # Channel pieces: cube via ACT square + DVE mul, then
                    # the on-device 2nd difference for the d-channels whose
                    # taps are fully cubed so far. GPSIMD is kept idle: any
                    # Pool op locks the SBUF port pair that DVE's 2-input
                    # ops need, fully blocking them. The kernel's first
                    # tile is split finest so the PE start latency is a
                    # quarter-tile of ACT+DVE work; steady-state tiles use
                    # two pieces (enough to overlap, fewest op overheads).
                    if half == 0 and it == 0:
                        pieces = [(0, 3), (3, 5), (5, NCA), (NCA, 9),
                                  (9, 11), (11, NCH)]
                    elif it == 1:
                        # the producers are still building their pipeline
                        # lead here; three pieces smooth the PE handoff
                        pieces = [(0, 5), (5, 10), (10, NCH)]
                    else:
                        pieces = [(0, NCA), (NCA, NCH)]
                    for (j0, j1) in pieces:
                        for j in range(j0, j1):
                            nc.scalar.activation(
                                c[:, j * HALF:(j + 1) * HALF], rt,
                                AF.Relu, bias=bias_tile[:, j:j + 1],
                                scale=-4.0)
                        w0, w1 = j0 * HALF, j1 * HALF
                        sq = sqpool.tile([128, w1 - w0], F32, tag="sq",
                                         name=f"sq_{half}_{it}_{j0}")
                        nc.scalar.activation(sq, c[:, w0:w1], AF.Square)
                        nc.vector.tensor_mul(c[:, w0:w1], sq, c[:, w0:w1])
                        # d_j = c_j - 2*c_{j+1} + c_{j+2} for j0-2 <= j < j1-2
                        d0 = max(0, j0 - 2) * HALF
                        d1 = (j1 - 2) * HALF
                        tmp = tpool.tile([128, d1 - d0], F32, tag="tA",
                                         name=f"tmp_{half}_{it}_{j0}")
                        nc.vector.scalar_tensor_tensor(
                            tmp, c[:, HALF + d0:HALF + d1], -2.0,
                            c[:, d0:d1], AluOpType.mult, AluOpType.add)
                        nc.vector.tensor_add(
                            d[:, d0:d1], tmp,
                            c[:, 2 * HALF + d0:2 * HALF + d1])
                    # silu for this tile's base matmul (emitted after the
                    # latency-critical squares; it==0 emitted it early)
                    if st is None:
                        st = spool.tile([128, HALF], F16, tag="silu")
                        nc.scalar.activation(st, xt_tiles[it], AF.Silu)
                    # d_12 = c_12 - 2*c_13 (c_14 == 0): single DVE op
                    # writing the fp16 channel directly
                    nc.vector.scalar_tensor_tensor(
                        d[:, W12:], c[:, HALF + W12:HALF + WD], -2.0,
                        c[:, W12:WD], AluOpType.mult, AluOpType.add)
                    # for the first tile of each half, run the silu base
                    # K-tile FIRST: silu is ready right after the x DMA,
                    # long before the first d chunk, so it covers the PE
                    # head (half 0) and the drain window (half 1)
                    silu_first = (it == 0)
                    if silu_first:
                        sbtt = wpool.tile([128, OUT_DIM], F16, tag="sbt")
                        nc.sync.dma_start(out=sbtt,
                                          in_=sbt[it * 128:(it + 1) * 128, :])
                        for oc in range(N_OC):
                            for m in range(M_TILES):
                                nc.tensor.matmul(
                                    psums[oc][m],
                                    lhsT=st[:, m * 128:m * 128 + 128],
                                    rhs=sbtt[:, oc * 512:(oc + 1) * 512],
                                    start=(kt_idx == 0),
                                    stop=False)
                        kt_idx += 1
                    # spline matmul K-tiles, chunk A first
                    for g in range(ND):
                        w2t = wpool.tile([128, OUT_DIM], F16, tag="w2")
                        row = (it * ND + g) * 128
                        nc.sync.dma_start(out=w2t, in_=w2[row:row + 128, :])
                        for oc in range(N_OC):
                            for m in range(M_TILES):
                                nc.tensor.matmul(
                                    psums[oc][m],
                                    lhsT=d[:, g * HALF + m * 128:
                                           g * HALF + m * 128 + 128],
                                    rhs=w2t[:, oc * 512:(oc + 1) * 512],
                                    start=(kt_idx == 0),
                                    stop=False)
                        kt_idx += 1
                        if g == NDA - 1 and it == 0 and pending_drain:
                            # previous half's PSUM drain, overlapped with
                            # this half's first tile of compute
                            emit_drains(*pending_drain)
                            pending_drain = None
                    # this tile's silu base matmul K-tile, accumulated into
                    # the same banks (uniform 14-K-tile PE cadence per tile)
                    if not silu_first:
                        sbtt = wpool.tile([128, OUT_DIM], F16, tag="sbt")
                        nc.sync.dma_start(out=sbtt,
                                          in_=sbt[it * 128:(it + 1) * 128, :])
                        for oc in range(N_OC):
                            for m in range(M_TILES):
                                nc.tensor.matmul(
                                    psums[oc][m],
                                    lhsT=st[:, m * 128:m * 128 + 128],
                                    rhs=sbtt[:, oc * 512:(oc + 1) * 512],
                                    start=False,
                                    stop=(kt_idx == n_kt - 1))
                        kt_idx += 1
                pending_drain = (psums, t0)
            emit_drains(*pending_drain)
    nc.compile()
    return nc


def _prepare_inputs(x, coeff, scale_base, scale_spline):
    x = np.asarray(x, dtype=np.float32)
    coeff = np.asarray(coeff, dtype=np.float32)
    scale_base = np.asarray(scale_base, dtype=np.float32)
    ss = float(np.asarray(scale_spline).reshape(-1)[0])
    # fold the outer 2nd difference (and the 1/6 from the 4th-difference
    # identity) into the weights: w2[o,i,j] = (w[j] - 2w[j-1] + w[j-2])/6
    w = coeff * (ss / 6.0)  # [OUT, IN, 11]
    wf = np.zeros((OUT_DIM, IN_DIM, ND), np.float32)
    wf[..., 0:11] += w
    wf[..., 1:12] += -2.0 * w
    wf[..., 2:13] += w
    # K-order: k = it*1664 + j*128 + p  ->  w2[k, o] = wf[o, it*128+p, j]
    w2 = wf.reshape(OUT_DIM, NIT, 128, ND)
    w2 = np.ascontiguousarray(w2.transpose(1, 3, 2, 0)).reshape(
        ND * IN_DIM, OUT_DIM)
    w2 = w2.astype(np.float16)
    sbt = np.ascontiguousarray(scale_base.T).astype(np.float16)
    in_maps = []
    for c in range(N_CORES):
        xt = np.ascontiguousarray(x[c * TPC:(c + 1) * TPC, :].T)
        in_maps.append({"xt": xt, "w2": w2, "sbt": sbt})
    return in_maps


def _get_bass():
    global _CACHED
    if _CACHED is None:
        _CACHED = _build_bass()
    return _CACHED


def run(inputs, trace=False, **kw):
    nc = _get_bass()
    in_maps = _prepare_inputs(inputs["x"], inputs["coeff"],
                              inputs["scale_base"], inputs["scale_spline"])
    res = run_bass_kernel_spmd(nc, in_maps, list(range(N_CORES)),
                               trace=trace, **kw)
    y = np.concatenate([np.asarray(res.results[c]["y"])
                        for c in range(N_CORES)], axis=0)
    return np.ascontiguousarray(y.astype(np.float32)), res


def kernel(x, grid, coeff, scale_base, scale_spline):
    y, _ = run({"x": x, "grid": grid, "coeff": coeff,
                "scale_base": scale_base, "scale_spline": scale_spline})
    return y
